# revision 11
# baseline (speedup 1.0000x reference)
"""GAT attention kernel for 8 trn2 NeuronCores (Bass/Tile), bf16 edition v3.

Math (restructured from the reference to avoid materializing h_j):
    wa1 = W @ a1, wa2 = W @ a2                      (device, once)
    s[n,k]  = x0[n]·wa1 + x[n,k]·wa2                (since h@a1 = x0@(W a1))
    e       = leaky_relu(s, 0.2)
    p       = exp(e) * adj                          (no max-sub: scores are small)
    att     = (p + EPS) / (sum_k p + 16*EPS)        (== uniform 1/16 when row fully masked,
                                                     matching reference softmax of all -9e15)
    xbar[n] = sum_k att[n,k] * x[n,k,:]
    out     = elu((xbar + x0) @ W)                  (since h_prime + h = (xbar + x0)@W)
    elu(z)  = relu(z) + exp(min(z,0)) - 1

Sharding: node dim N padded 50000 -> 50176 = 8 cores * 49 tiles * 128 rows.
Per 128-row tile the 2048 (n,k) pairs form 16 blocks of [128 nk-rows, 128 feat]
held as x_tile[:, b*128:(b+1)*128] in bf16 (host pre-permutes + casts so the
DMA is a single contiguous ~561KB transfer per tile).

DVE has a ~151-cycle fixed cost per instruction, so scores are computed with
TWO big ops instead of 17 accumulate-STTs: one tensor_tensor product against
a precomposed [128, 17*128] replicated-weights tile (16x wa2 then wa1, so the
x0.wa1 dot rides along as block 17), then one segmented tensor_reduce(axis=X)
[128,17,128] -> [128,17] in bf16 (2x 16-bit DVE path).

Per tile:
  DVE : prod TT, segmented reduce, att = (p+eps)*RZ, attseg = SEGBIG*att,
        recip_fast, y = r-1+e
  PE  : si scatter (Cm fp32), Z group-sum (SEG), RZrep (E8), x0^T identity
        matmul + 16 bf16 xbar matmuls (accumulate xbarT in PSUM), final
  ACT : si f32 cast, si_s copy, Prelu(0.2), Exp, tz copy, ST copy,
        elu pieces r=relu(z), t=relu(-z), e=exp(-t)
  GPS : Dt (si broadcast mask), s2 = s+si_s, p = ex*adj
"""

import numpy as np

N, K, F = 50000, 16, 128
ALPHA = 0.2
NCORES = 8
TILE = 128
NTILES = 49
RPC = TILE * NTILES          # rows per core = 6272
BPT = K                      # nk-blocks per tile = 16
XCOLS = BPT * F + F + K      # x blocks + x0 + adj(s-layout) = 2192
EPS = 1e-12
WREP = (BPT + 1) * F         # replicated weights row: 16x wa2 + wa1 = 2176

_NC_CACHE = {}


def _consts_np():
    p = np.arange(128)
    j8 = np.arange(8)
    b16 = np.arange(16)
    ident = np.eye(128, dtype=np.float32)
    ones = np.ones((128, 128), dtype=np.float32)
    # C[n, q] = 1 iff n%8 == q//16   (si scatter: out[q,b] = si[8b + q//16])
    Cm = (p[:, None] % 8 == p[None, :] // 16).astype(np.float32)
    # SEG[q, j] = 1 iff q//16 == j   [128, 8]
    seg = (p[:, None] // 16 == j8[None, :]).astype(np.float32)
    # E8 rows 0..8: E8[j, q] = 1 iff q//16 == j (used as lhsT [8,128])
    e8 = ((p[:, None] < 8) & (p[None, :] // 16 == p[:, None])).astype(np.float32)
    # SEGBIG[q, 8b+j] = 1 iff j == q//16  (pattern repeats over b)
    segbig = (p[:, None] // 16 == (p[None, :] % 8)).astype(np.float32)
    # SEG8[n, b] = 1 iff n//8 == b   [128, 16]
    seg8 = (p[:, None] // 8 == b16[None, :]).astype(np.float32)
    return np.concatenate([ident, ones, Cm, seg, e8, segbig, seg8], axis=1)


def _consts_full_np(W, a):
    # consts + W + a1 + a2 packed into one f32 tensor -> one setup DMA
    return np.ascontiguousarray(
        np.concatenate(
            [_consts_np(), W.astype(np.float32),
             a[:F].astype(np.float32), a[F:].astype(np.float32)], axis=1)
    )  # [128, 664+128+2] = [128, 794]


def _build_nc(ntiles=NTILES, finalize=True):
    import concourse.mybir as mybir
    import concourse.tile as tile
    from concourse import bacc

    fp = mybir.dt.float32
    bf = mybir.dt.bfloat16
    AF = mybir.ActivationFunctionType
    OP = mybir.AluOpType

    nc = bacc.Bacc("TRN2")
    xd = nc.dram_tensor("xd", [ntiles, 128, XCOLS], bf, kind="ExternalInput")
    cst = nc.dram_tensor("cst", [128, 794], fp, kind="ExternalInput")
    yd = nc.dram_tensor("yd", [ntiles, 128, F], bf, kind="ExternalOutput")

    with tile.TileContext(nc) as tc:
        with (
            tc.tile_pool(name="const", bufs=1) as constp,
            tc.tile_pool(name="xin", bufs=9) as xin,
            tc.tile_pool(name="small", bufs=4) as small,
            tc.tile_pool(name="big", bufs=4) as big,
            tc.tile_pool(name="scrp", bufs=2) as scrp,
            tc.tile_pool(name="yout", bufs=3) as yout,
            tc.tile_pool(name="ps", bufs=1, space="PSUM") as ps,
        ):
            # ---------------- setup (single DMA -> single wait chains) ----
            consts = constp.tile([128, 794], fp)
            nc.sync.dma_start(out=consts, in_=cst[:, :])
            IDENT = consts[:, 0:128]
            ONES = consts[:, 128:256]
            Cm = consts[:, 256:384]
            SEG = consts[:, 384:392]
            E8 = consts[:, 392:520]
            SEGBIG = consts[:, 520:648]
            SEG8 = consts[:, 648:664]
            W_sb = consts[:, 664:792]
            a1_sb = consts[:, 792:793]
            a2_sb = consts[:, 793:794]

            # W^T via identity matmul (fp32, one-time)
            WT_ps = ps.tile([128, 128], fp, tag="mm", bufs=3)
            nc.tensor.matmul(WT_ps, lhsT=W_sb, rhs=IDENT, start=True, stop=True)
            WT_sb = constp.tile([128, 128], fp)
            nc.scalar.activation(out=WT_sb, in_=WT_ps, func=AF.Copy)

            # bf16 copies of W and the identity
            W_bf = constp.tile([128, 128], bf, tag="wbf")
            nc.scalar.activation(out=W_bf, in_=W_sb, func=AF.Copy)
            IDENT_bf = constp.tile([128, 128], bf, tag="identbf")
            nc.scalar.activation(out=IDENT_bf, in_=IDENT, func=AF.Copy)
            SEGBIG_bf = constp.tile([128, 128], bf, tag="segbigbf")
            nc.scalar.activation(out=SEGBIG_bf, in_=SEGBIG, func=AF.Copy)

            # wa1 = W@a1, wa2 = W@a2 as columns
            wa_ps = ps.tile([128, 2], fp, tag="si", bufs=1)
            nc.tensor.matmul(wa_ps[:, 0:1], lhsT=WT_sb, rhs=a1_sb, start=True, stop=True)
            nc.tensor.matmul(wa_ps[:, 1:2], lhsT=WT_sb, rhs=a2_sb, start=True, stop=True)
            wa_cols = constp.tile([128, 2], fp)
            nc.scalar.activation(out=wa_cols, in_=wa_ps, func=AF.Copy)

            # warep_cat [128, 17*128] bf16: 16 copies of wa2 broadcast rows,
            # then one of wa1 (so one TT+reduce covers scores and si)
            warep_cat = constp.tile([128, WREP], bf, tag="warep")
            for i in (0, 1):
                row_ps = ps.tile([1, 128], fp, tag="Z", bufs=2)
                nc.tensor.matmul(row_ps, lhsT=wa_cols[:, i:i + 1], rhs=IDENT,
                                 start=True, stop=True)
                row_sb = constp.tile([1, 128], fp, tag=f"warow{i}")
                nc.scalar.activation(out=row_sb, in_=row_ps, func=AF.Copy)
                rep_ps = ps.tile([128, 128], fp, tag="mm", bufs=3)
                nc.tensor.matmul(rep_ps, lhsT=ONES[0:1, :], rhs=row_sb[0:1, :],
                                 start=True, stop=True)
                if i == 1:
                    # wa2 broadcast fills the 16 neighbor-score blocks
                    for b in range(BPT):
                        nc.scalar.activation(
                            out=warep_cat[:, b * F:(b + 1) * F],
                            in_=rep_ps, func=AF.Copy)
                else:
                    # wa1 broadcast is block 17 (the x0.wa1 ride-along)
                    nc.scalar.activation(
                        out=warep_cat[:, BPT * F:WREP],
                        in_=rep_ps, func=AF.Copy)

            # ---------------- software-pipelined tile loop ----------------
            # load(t) | score(t-2) | mask(t-3) | recip(t-4) | att/xbar(t-5)
            # | final/elu/store(t-6)
            st = {}

            def phase_load(t):
                xall = xin.tile([128, XCOLS], bf, tag="x")
                nc.sync.dma_start(out=xall, in_=xd[t])
                st[t] = {"xall": xall}

            def phase_score(t):
                d = st[t]
                xall = d["xall"]
                # one big product + one segmented reduce: s17[:, 0:16] are
                # the 16 neighbor scores, s17[:, 16] is si = x0.wa1
                scr = scrp.tile([128, WREP], bf, tag="scr")
                nc.vector.tensor_mul(out=scr, in0=xall[:, 0:WREP], in1=warep_cat)
                s17 = small.tile([128, BPT + 1], fp, tag="s17")
                nc.vector.tensor_reduce(
                    out=s17,
                    in_=scr.rearrange("p (b f) -> p b f", f=F),
                    axis=mybir.AxisListType.X, op=OP.add,
                )
                Dt = small.tile([128, K], fp, tag="D")
                si_bc = s17[:, BPT:BPT + 1].rearrange(
                    "p (b o) -> p b o", o=1).to_broadcast([128, K, 1])
                nc.gpsimd.tensor_mul(
                    out=Dt.rearrange("p (b o) -> p b o", o=1),
                    in0=SEG8.rearrange("p (b o) -> p b o", o=1), in1=si_bc)
                si_ps = ps.tile([128, K], fp, tag="si", bufs=1)
                nc.tensor.matmul(si_ps, lhsT=Cm, rhs=Dt, start=True, stop=True)
                si_s = small.tile([128, K], fp, tag="si_s")
                nc.scalar.activation(out=si_s, in_=si_ps, func=AF.Copy)
                s2 = small.tile([128, K], fp, tag="s2")
                nc.gpsimd.tensor_add(out=s2, in0=s17[:, 0:BPT], in1=si_s)
                d["s2"] = s2

            def phase_mask(t):
                d = st[t]
                adj_f = d["xall"][:, BPT * F + F:XCOLS]
                ls = small.tile([128, K], fp, tag="ls")
                nc.scalar.activation(out=ls, in_=d["s2"], func=AF.Prelu,
                                     alpha=ALPHA)
                exp_s = small.tile([128, K], fp, tag="exp_s")
                nc.scalar.activation(out=exp_s, in_=ls, func=AF.Exp)
                p_s = small.tile([128, K], fp, tag="p_s")
                nc.gpsimd.tensor_mul(out=p_s, in0=exp_s, in1=adj_f)
                Z_ps = ps.tile([8, K], fp, tag="Z", bufs=2)
                nc.tensor.matmul(Z_ps, lhsT=SEG, rhs=p_s, start=True, stop=True)
                tz = small.tile([8, K], fp, tag="tz")
                nc.scalar.activation(out=tz, in_=Z_ps, func=AF.Copy, bias=16.0 * EPS)
                d["p_s"] = p_s
                d["tz"] = tz

            def phase_recip(t):
                d = st[t]
                RZ = small.tile([8, K], fp, tag="RZ")
                nc.vector.reciprocal_approx_fast(RZ, d["tz"])
                RZrep_ps = ps.tile([128, K], fp, tag="RZrep", bufs=2)
                nc.tensor.matmul(RZrep_ps, lhsT=E8[0:8, :], rhs=RZ,
                                 start=True, stop=True)
                d["RZrep"] = RZrep_ps

            def phase_xbar(t):
                d = st[t]
                xall = d["xall"]
                att = small.tile([128, K], bf, tag="att")
                nc.vector.scalar_tensor_tensor(
                    out=att, in0=d["p_s"], scalar=EPS, in1=d["RZrep"],
                    op0=OP.add, op1=OP.mult,
                )
                attseg = big.tile([128, 128], bf, tag="attseg")
                att_bc = att.rearrange("p (b o) -> p b o", o=1).to_broadcast([128, K, 8])
                nc.vector.tensor_mul(
                    out=attseg.rearrange("p (b j) -> p b j", j=8),
                    in0=SEGBIG_bf.rearrange("p (b j) -> p b j", j=8),
                    in1=att_bc,
                )
                xbarT_ps = ps.tile([128, 128], fp, tag="mm", bufs=3)
                nc.tensor.matmul(xbarT_ps, lhsT=xall[:, BPT * F:BPT * F + F],
                                 rhs=IDENT_bf, start=True, stop=False,
                                 skip_group_check=True)
                for b in range(BPT):
                    nc.tensor.matmul(
                        xbarT_ps[:, 8 * b:8 * b + 8],
                        lhsT=xall[:, b * F:(b + 1) * F],
                        rhs=attseg[:, 8 * b:8 * b + 8],
                        start=False, stop=(b == BPT - 1),
                        skip_group_check=True,
                    )
                d["xbarT"] = xbarT_ps

            def phase_out(t):
                d = st[t]
                ST_sb = big.tile([128, 128], bf, tag="ST")
                nc.scalar.activation(out=ST_sb, in_=d["xbarT"], func=AF.Copy)
                zfin_ps = ps.tile([128, 128], fp, tag="mm", bufs=3)
                nc.tensor.matmul(zfin_ps, lhsT=ST_sb, rhs=W_bf, start=True, stop=True)
                r_sb = big.tile([128, 128], bf, tag="r")
                nc.scalar.activation(out=r_sb, in_=zfin_ps, func=AF.Relu)
                e_sb = big.tile([128, 128], bf, tag="e")
                nc.scalar.activation(out=e_sb, in_=zfin_ps, func=AF.Exp)
                d["r"] = r_sb
                d["e"] = e_sb

            def phase_store(t):
                d = st[t]
                y_sb = yout.tile([128, 128], bf, tag="y")
                nc.vector.scalar_tensor_tensor(
                    out=y_sb, in0=d["e"], scalar=-1.0, in1=d["r"],
                    op0=OP.add, op1=OP.min,
                )
                nc.sync.dma_start(out=yd[t], in_=y_sb)
                del st[t]

            for r in range(ntiles + 7):
                if r < ntiles:
                    phase_load(r)
                if 0 <= r - 2 < ntiles:
                    phase_score(r - 2)
                if 0 <= r - 3 < ntiles:
                    phase_mask(r - 3)
                if 0 <= r - 4 < ntiles:
                    phase_recip(r - 4)
                if 0 <= r - 5 < ntiles:
                    phase_xbar(r - 5)
                if 0 <= r - 6 < ntiles:
                    phase_out(r - 6)
                if 0 <= r - 7 < ntiles:
                    phase_store(r - 7)

    if finalize:
        nc.finalize()
    return nc


def _get_nc(ntiles=NTILES):
    if ntiles not in _NC_CACHE:
        _NC_CACHE[ntiles] = _build_nc(ntiles)
    return _NC_CACHE[ntiles]


def _shard_inputs(orignal_x, x, adj, W, a, ncores=NCORES, ntiles=NTILES):
    import ml_dtypes
    bf16 = ml_dtypes.bfloat16
    f32 = np.float32
    rpc = TILE * ntiles
    n_used = rpc * ncores
    x = np.asarray(x, f32)
    x0 = np.asarray(orignal_x, f32)
    adj = np.asarray(adj, np.int32)
    consts = _consts_full_np(np.asarray(W, f32), np.asarray(a, f32))
    n = x.shape[0]

    in_maps = []
    for c in range(ncores):
        lo = c * rpc
        hi = min((c + 1) * rpc, n)
        rows = hi - lo
        xc = x[lo:hi]
        x0c = x0[lo:hi]
        adjc = adj[lo:hi]
        if rows < rpc:
            pad = rpc - rows
            xc = np.concatenate([xc, np.zeros((pad, K, F), f32)])
            x0c = np.concatenate([x0c, np.zeros((pad, F), f32)])
            adjc = np.concatenate([adjc, np.zeros((pad, K), np.int32)])
        # per-tile layout [t, q, b*F+f] (s-layout blocks) with x0 natural
        # and bf16 adj (s-layout) packed as trailing cols
        xdev = np.empty((ntiles, 128, XCOLS), bf16)
        xdev[:, :, :BPT * F] = xc.reshape(ntiles, BPT, 128, F).transpose(
            0, 2, 1, 3).reshape(ntiles, 128, BPT * F).astype(bf16)
        xdev[:, :, BPT * F:BPT * F + F] = x0c.reshape(ntiles, 128, F).astype(bf16)
        xdev[:, :, BPT * F + F:] = adjc.reshape(ntiles, BPT, 128).transpose(
            0, 2, 1).astype(bf16)
        in_maps.append({
            "xd": xdev,
            "cst": consts,
        })
    assert n <= n_used
    return in_maps


_LAST_RESULTS = None


def kernel(orignal_x, x, adj, W, a):
    import os
    os.environ.setdefault("JAX_PLATFORMS", "")
    from concourse.bass_utils import run_bass_kernel_spmd

    global _LAST_RESULTS
    nc = _get_nc()
    in_maps = _shard_inputs(orignal_x, x, adj, W, a)
    res = run_bass_kernel_spmd(nc, in_maps, list(range(NCORES)))
    _LAST_RESULTS = res
    y = np.concatenate(
        [np.asarray(r["yd"], np.float32).reshape(RPC, F) for r in res.results],
        axis=0)
    return np.ascontiguousarray(y[:N])


# revision 12
# speedup vs baseline: 1.0140x; 1.0140x over previous
"""GAT attention kernel for 8 trn2 NeuronCores (Bass/Tile), bf16 edition v3.

Math (restructured from the reference to avoid materializing h_j):
    wa1 = W @ a1, wa2 = W @ a2                      (device, once)
    s[n,k]  = x0[n]·wa1 + x[n,k]·wa2                (since h@a1 = x0@(W a1))
    e       = leaky_relu(s, 0.2)
    p       = exp(e) * adj                          (no max-sub: scores are small)
    att     = (p + EPS) / (sum_k p + 16*EPS)        (== uniform 1/16 when row fully masked,
                                                     matching reference softmax of all -9e15)
    xbar[n] = sum_k att[n,k] * x[n,k,:]
    out     = elu((xbar + x0) @ W)                  (since h_prime + h = (xbar + x0)@W)
    elu(z)  = relu(z) + exp(min(z,0)) - 1

Sharding: node dim N padded 50000 -> 50176 = 8 cores * 49 tiles * 128 rows.
Per 128-row tile the 2048 (n,k) pairs form 16 blocks of [128 nk-rows, 128 feat]
held as x_tile[:, b*128:(b+1)*128] in bf16 (host pre-permutes + casts so the
DMA is a single contiguous ~561KB transfer per tile).

DVE has a ~151-cycle fixed cost per instruction, so scores are computed with
TWO big ops instead of 17 accumulate-STTs: one tensor_tensor product against
a precomposed [128, 17*128] replicated-weights tile (16x wa2 then wa1, so the
x0.wa1 dot rides along as block 17), then one segmented tensor_reduce(axis=X)
[128,17,128] -> [128,17] in bf16 (2x 16-bit DVE path).

Per tile:
  DVE : prod TT, segmented reduce, att = (p+eps)*RZ, attseg = SEGBIG*att,
        recip_fast, y = r-1+e
  PE  : si scatter (Cm fp32), Z group-sum (SEG), RZrep (E8), x0^T identity
        matmul + 16 bf16 xbar matmuls (accumulate xbarT in PSUM), final
  ACT : si f32 cast, si_s copy, Prelu(0.2), Exp, tz copy, ST copy,
        elu pieces r=relu(z), t=relu(-z), e=exp(-t)
  GPS : Dt (si broadcast mask), s2 = s+si_s, p = ex*adj
"""

import numpy as np

N, K, F = 50000, 16, 128
ALPHA = 0.2
NCORES = 8
TILE = 128
NTILES = 49
RPC = TILE * NTILES          # rows per core = 6272
BPT = K                      # nk-blocks per tile = 16
XCOLS = BPT * F + F + K      # x blocks + x0 + adj(s-layout) = 2192
EPS = 1e-12
WREP = (BPT + 1) * F         # replicated weights row: 16x wa2 + wa1 = 2176

_NC_CACHE = {}


def _consts_np():
    p = np.arange(128)
    j8 = np.arange(8)
    b16 = np.arange(16)
    ident = np.eye(128, dtype=np.float32)
    ones = np.ones((128, 128), dtype=np.float32)
    # C[n, q] = 1 iff n%8 == q//16   (si scatter: out[q,b] = si[8b + q//16])
    Cm = (p[:, None] % 8 == p[None, :] // 16).astype(np.float32)
    # SEG[q, j] = 1 iff q//16 == j   [128, 8]
    seg = (p[:, None] // 16 == j8[None, :]).astype(np.float32)
    # E8 rows 0..8: E8[j, q] = 1 iff q//16 == j (used as lhsT [8,128])
    e8 = ((p[:, None] < 8) & (p[None, :] // 16 == p[:, None])).astype(np.float32)
    # SEGBIG[q, 8b+j] = 1 iff j == q//16  (pattern repeats over b)
    segbig = (p[:, None] // 16 == (p[None, :] % 8)).astype(np.float32)
    # SEG8[n, b] = 1 iff n//8 == b   [128, 16]
    seg8 = (p[:, None] // 8 == b16[None, :]).astype(np.float32)
    return np.concatenate([ident, ones, Cm, seg, e8, segbig, seg8], axis=1)


def _consts_full_np(W, a):
    # consts + W + a1 + a2 packed into one f32 tensor -> one setup DMA
    return np.ascontiguousarray(
        np.concatenate(
            [_consts_np(), W.astype(np.float32),
             a[:F].astype(np.float32), a[F:].astype(np.float32)], axis=1)
    )  # [128, 664+128+2] = [128, 794]


def _build_nc(ntiles=NTILES, finalize=True):
    import concourse.mybir as mybir
    import concourse.tile as tile
    from concourse import bacc

    fp = mybir.dt.float32
    bf = mybir.dt.bfloat16
    AF = mybir.ActivationFunctionType
    OP = mybir.AluOpType

    nc = bacc.Bacc("TRN2")
    xd = nc.dram_tensor("xd", [ntiles, 128, XCOLS], bf, kind="ExternalInput")
    cst = nc.dram_tensor("cst", [128, 794], fp, kind="ExternalInput")
    yd = nc.dram_tensor("yd", [ntiles, 128, F], bf, kind="ExternalOutput")

    with tile.TileContext(nc) as tc:
        with (
            tc.tile_pool(name="const", bufs=1) as constp,
            tc.tile_pool(name="xin", bufs=9) as xin,
            tc.tile_pool(name="small", bufs=4) as small,
            tc.tile_pool(name="big", bufs=4) as big,
            tc.tile_pool(name="scrp", bufs=2) as scrp,
            tc.tile_pool(name="yout", bufs=3) as yout,
            tc.tile_pool(name="ps", bufs=1, space="PSUM") as ps,
        ):
            # ---------------- setup (single DMA -> single wait chains) ----
            consts = constp.tile([128, 794], fp)
            nc.sync.dma_start(out=consts, in_=cst[:, :])
            IDENT = consts[:, 0:128]
            ONES = consts[:, 128:256]
            Cm = consts[:, 256:384]
            SEG = consts[:, 384:392]
            E8 = consts[:, 392:520]
            SEGBIG = consts[:, 520:648]
            SEG8 = consts[:, 648:664]
            W_sb = consts[:, 664:792]
            a1_sb = consts[:, 792:793]
            a2_sb = consts[:, 793:794]

            # W^T via identity matmul (fp32, one-time)
            WT_ps = ps.tile([128, 128], fp, tag="mm", bufs=3)
            nc.tensor.matmul(WT_ps, lhsT=W_sb, rhs=IDENT, start=True, stop=True)
            WT_sb = constp.tile([128, 128], fp)
            nc.scalar.activation(out=WT_sb, in_=WT_ps, func=AF.Copy)

            # bf16 copies of W and the identity
            W_bf = constp.tile([128, 128], bf, tag="wbf")
            nc.scalar.activation(out=W_bf, in_=W_sb, func=AF.Copy)
            IDENT_bf = constp.tile([128, 128], bf, tag="identbf")
            nc.scalar.activation(out=IDENT_bf, in_=IDENT, func=AF.Copy)
            SEGBIG_bf = constp.tile([128, 128], bf, tag="segbigbf")
            nc.scalar.activation(out=SEGBIG_bf, in_=SEGBIG, func=AF.Copy)

            # wa1 = W@a1, wa2 = W@a2 as columns
            wa_ps = ps.tile([128, 2], fp, tag="si", bufs=1)
            nc.tensor.matmul(wa_ps[:, 0:1], lhsT=WT_sb, rhs=a1_sb, start=True, stop=True)
            nc.tensor.matmul(wa_ps[:, 1:2], lhsT=WT_sb, rhs=a2_sb, start=True, stop=True)
            wa_cols = constp.tile([128, 2], fp)
            nc.scalar.activation(out=wa_cols, in_=wa_ps, func=AF.Copy)

            # warep_cat [128, 17*128] bf16: 16 copies of wa2 broadcast rows,
            # then one of wa1 (so one TT+reduce covers scores and si)
            warep_cat = constp.tile([128, WREP], bf, tag="warep")
            for i in (0, 1):
                row_ps = ps.tile([1, 128], fp, tag="Z", bufs=2)
                nc.tensor.matmul(row_ps, lhsT=wa_cols[:, i:i + 1], rhs=IDENT,
                                 start=True, stop=True)
                row_sb = constp.tile([1, 128], fp, tag=f"warow{i}")
                nc.scalar.activation(out=row_sb, in_=row_ps, func=AF.Copy)
                rep_ps = ps.tile([128, 128], fp, tag="mm", bufs=3)
                nc.tensor.matmul(rep_ps, lhsT=ONES[0:1, :], rhs=row_sb[0:1, :],
                                 start=True, stop=True)
                if i == 1:
                    # wa2 broadcast fills the 16 neighbor-score blocks in one
                    # strided-broadcast copy
                    rep_bc = rep_ps.rearrange(
                        "p (o f) -> p o f", o=1).to_broadcast([128, BPT, F])
                    nc.scalar.activation(
                        out=warep_cat[:, 0:BPT * F].rearrange(
                            "p (b f) -> p b f", f=F),
                        in_=rep_bc, func=AF.Copy)
                else:
                    # wa1 broadcast is block 17 (the x0.wa1 ride-along)
                    nc.scalar.activation(
                        out=warep_cat[:, BPT * F:WREP],
                        in_=rep_ps, func=AF.Copy)

            # ---------------- software-pipelined tile loop ----------------
            # load(t) | score(t-2) | mask(t-3) | recip(t-4) | att/xbar(t-5)
            # | final/elu/store(t-6)
            st = {}

            def phase_load(t):
                xall = xin.tile([128, XCOLS], bf, tag="x")
                nc.sync.dma_start(out=xall, in_=xd[t])
                st[t] = {"xall": xall}

            def phase_score(t):
                d = st[t]
                xall = d["xall"]
                # one big product + one segmented reduce: s17[:, 0:16] are
                # the 16 neighbor scores, s17[:, 16] is si = x0.wa1
                scr = scrp.tile([128, WREP], bf, tag="scr")
                nc.vector.tensor_mul(out=scr, in0=xall[:, 0:WREP], in1=warep_cat)
                s17 = small.tile([128, BPT + 1], fp, tag="s17")
                nc.vector.tensor_reduce(
                    out=s17,
                    in_=scr.rearrange("p (b f) -> p b f", f=F),
                    axis=mybir.AxisListType.X, op=OP.add,
                )
                Dt = small.tile([128, K], fp, tag="D")
                si_bc = s17[:, BPT:BPT + 1].rearrange(
                    "p (b o) -> p b o", o=1).to_broadcast([128, K, 1])
                nc.gpsimd.tensor_mul(
                    out=Dt.rearrange("p (b o) -> p b o", o=1),
                    in0=SEG8.rearrange("p (b o) -> p b o", o=1), in1=si_bc)
                si_ps = ps.tile([128, K], fp, tag="si", bufs=1)
                nc.tensor.matmul(si_ps, lhsT=Cm, rhs=Dt, start=True, stop=True)
                si_s = small.tile([128, K], fp, tag="si_s")
                nc.scalar.activation(out=si_s, in_=si_ps, func=AF.Copy)
                s2 = small.tile([128, K], fp, tag="s2")
                nc.gpsimd.tensor_add(out=s2, in0=s17[:, 0:BPT], in1=si_s)
                d["s2"] = s2

            def phase_mask(t):
                d = st[t]
                adj_f = d["xall"][:, BPT * F + F:XCOLS]
                ls = small.tile([128, K], fp, tag="ls")
                nc.scalar.activation(out=ls, in_=d["s2"], func=AF.Prelu,
                                     alpha=ALPHA)
                exp_s = small.tile([128, K], fp, tag="exp_s")
                nc.scalar.activation(out=exp_s, in_=ls, func=AF.Exp)
                p_s = small.tile([128, K], fp, tag="p_s")
                nc.gpsimd.tensor_mul(out=p_s, in0=exp_s, in1=adj_f)
                Z_ps = ps.tile([8, K], fp, tag="Z", bufs=2)
                nc.tensor.matmul(Z_ps, lhsT=SEG, rhs=p_s, start=True, stop=True)
                tz = small.tile([8, K], fp, tag="tz")
                nc.scalar.activation(out=tz, in_=Z_ps, func=AF.Copy, bias=16.0 * EPS)
                d["p_s"] = p_s
                d["tz"] = tz

            def phase_recip(t):
                d = st[t]
                RZ = small.tile([8, K], fp, tag="RZ")
                nc.vector.reciprocal_approx_fast(RZ, d["tz"])
                RZrep_ps = ps.tile([128, K], fp, tag="RZrep", bufs=2)
                nc.tensor.matmul(RZrep_ps, lhsT=E8[0:8, :], rhs=RZ,
                                 start=True, stop=True)
                d["RZrep"] = RZrep_ps

            def phase_xbar(t):
                d = st[t]
                xall = d["xall"]
                att = small.tile([128, K], bf, tag="att")
                nc.vector.scalar_tensor_tensor(
                    out=att, in0=d["p_s"], scalar=EPS, in1=d["RZrep"],
                    op0=OP.add, op1=OP.mult,
                )
                attseg = big.tile([128, 128], bf, tag="attseg")
                att_bc = att.rearrange("p (b o) -> p b o", o=1).to_broadcast([128, K, 8])
                nc.vector.tensor_mul(
                    out=attseg.rearrange("p (b j) -> p b j", j=8),
                    in0=SEGBIG_bf.rearrange("p (b j) -> p b j", j=8),
                    in1=att_bc,
                )
                xbarT_ps = ps.tile([128, 128], fp, tag="mm", bufs=3)
                nc.tensor.matmul(xbarT_ps, lhsT=xall[:, BPT * F:BPT * F + F],
                                 rhs=IDENT_bf, start=True, stop=False,
                                 skip_group_check=True)
                for b in range(BPT):
                    nc.tensor.matmul(
                        xbarT_ps[:, 8 * b:8 * b + 8],
                        lhsT=xall[:, b * F:(b + 1) * F],
                        rhs=attseg[:, 8 * b:8 * b + 8],
                        start=False, stop=(b == BPT - 1),
                        skip_group_check=True,
                    )
                d["xbarT"] = xbarT_ps

            def phase_out(t):
                d = st[t]
                ST_sb = big.tile([128, 128], bf, tag="ST")
                nc.scalar.activation(out=ST_sb, in_=d["xbarT"], func=AF.Copy)
                zfin_ps = ps.tile([128, 128], fp, tag="mm", bufs=3)
                nc.tensor.matmul(zfin_ps, lhsT=ST_sb, rhs=W_bf, start=True, stop=True)
                r_sb = big.tile([128, 128], bf, tag="r")
                nc.scalar.activation(out=r_sb, in_=zfin_ps, func=AF.Relu)
                e_sb = big.tile([128, 128], bf, tag="e")
                nc.scalar.activation(out=e_sb, in_=zfin_ps, func=AF.Exp)
                d["r"] = r_sb
                d["e"] = e_sb

            def phase_store(t):
                d = st[t]
                y_sb = yout.tile([128, 128], bf, tag="y")
                nc.vector.scalar_tensor_tensor(
                    out=y_sb, in0=d["e"], scalar=-1.0, in1=d["r"],
                    op0=OP.add, op1=OP.min,
                )
                nc.sync.dma_start(out=yd[t], in_=y_sb)
                del st[t]

            for r in range(ntiles + 7):
                if r < ntiles:
                    phase_load(r)
                if 0 <= r - 2 < ntiles:
                    phase_score(r - 2)
                if 0 <= r - 3 < ntiles:
                    phase_mask(r - 3)
                if 0 <= r - 4 < ntiles:
                    phase_recip(r - 4)
                if 0 <= r - 5 < ntiles:
                    phase_xbar(r - 5)
                if 0 <= r - 6 < ntiles:
                    phase_out(r - 6)
                if 0 <= r - 7 < ntiles:
                    phase_store(r - 7)

    if finalize:
        nc.finalize()
    return nc


def _get_nc(ntiles=NTILES):
    if ntiles not in _NC_CACHE:
        _NC_CACHE[ntiles] = _build_nc(ntiles)
    return _NC_CACHE[ntiles]


def _shard_inputs(orignal_x, x, adj, W, a, ncores=NCORES, ntiles=NTILES):
    import ml_dtypes
    bf16 = ml_dtypes.bfloat16
    f32 = np.float32
    rpc = TILE * ntiles
    n_used = rpc * ncores
    x = np.asarray(x, f32)
    x0 = np.asarray(orignal_x, f32)
    adj = np.asarray(adj, np.int32)
    consts = _consts_full_np(np.asarray(W, f32), np.asarray(a, f32))
    n = x.shape[0]

    in_maps = []
    for c in range(ncores):
        lo = c * rpc
        hi = min((c + 1) * rpc, n)
        rows = hi - lo
        xc = x[lo:hi]
        x0c = x0[lo:hi]
        adjc = adj[lo:hi]
        if rows < rpc:
            pad = rpc - rows
            xc = np.concatenate([xc, np.zeros((pad, K, F), f32)])
            x0c = np.concatenate([x0c, np.zeros((pad, F), f32)])
            adjc = np.concatenate([adjc, np.zeros((pad, K), np.int32)])
        # per-tile layout [t, q, b*F+f] (s-layout blocks) with x0 natural
        # and bf16 adj (s-layout) packed as trailing cols
        xdev = np.empty((ntiles, 128, XCOLS), bf16)
        xdev[:, :, :BPT * F] = xc.reshape(ntiles, BPT, 128, F).transpose(
            0, 2, 1, 3).reshape(ntiles, 128, BPT * F).astype(bf16)
        xdev[:, :, BPT * F:BPT * F + F] = x0c.reshape(ntiles, 128, F).astype(bf16)
        xdev[:, :, BPT * F + F:] = adjc.reshape(ntiles, BPT, 128).transpose(
            0, 2, 1).astype(bf16)
        in_maps.append({
            "xd": xdev,
            "cst": consts,
        })
    assert n <= n_used
    return in_maps


_LAST_RESULTS = None


def kernel(orignal_x, x, adj, W, a):
    import os
    os.environ.setdefault("JAX_PLATFORMS", "")
    from concourse.bass_utils import run_bass_kernel_spmd

    global _LAST_RESULTS
    nc = _get_nc()
    in_maps = _shard_inputs(orignal_x, x, adj, W, a)
    res = run_bass_kernel_spmd(nc, in_maps, list(range(NCORES)))
    _LAST_RESULTS = res
    y = np.concatenate(
        [np.asarray(r["yd"], np.float32).reshape(RPC, F) for r in res.results],
        axis=0)
    return np.ascontiguousarray(y[:N])
